# revision 27
# baseline (speedup 1.0000x reference)
"""Trainium2 Bass kernel for the fine-grained caption/image sparse-attention
similarity module.

Math (per image i, caption-word row x = (c,w)):
    q = LN(caps;g1,b1)@Wq^T + bq          -> folded: LNraw@WqgT + qb
    k = LN(imgs;g2,b2)@Wk^T + bk          -> folded: LNraw@WkgT + bk'
    v = LN(imgs;g3,b3)@Wv^T + bv          -> folded: LNraw@WvgT + bv'
    e[x,i,r]  = exp(q.k/sqrt(D)) * mask01[i,r]        (unnormalized attn)
    ssum      = sum_r e;  a = e/ssum                  (true attn)
    ctx       = a@V_i  (rank-36!)  ->  never materialized:
      V'_i = V_i - rowmean(V_i)   (row-centered -> LN centering is exact)
      P_i  = V'_i @ (Wo*g4)^T
      S_i  = V'_i V'_i^T / D,  M_i = P_i P_i^T        (36x36 Grams)
    g   = sqrt(e^T S_i e + LN_EPS*ssum^2)             ( = ssum*sqrt(var+eps) )
    num = e.(Q@P_i^T) ;  nM = e^T M_i e
    s   = num / (sqrt(nM) + EPS*g)                    (== cos-sim numerically)
    out[i, x] = s*capm + (capm-1)                     (invalid cap words -> -1)

Sharding: 8 images per core (replicated captions/weights), no collectives.
"""

import os
import sys

import numpy as np

EPS = 1e-8
LN_EPS = 1e-5

Bi, R, Bc, W, D = 64, 36, 64, 40, 512
N_CORES = 8
BI_S = Bi // N_CORES      # 8 images per core
IR = BI_S * R             # 288
NCW = Bc * W              # 2560 caption-word rows
NT = NCW // 128           # 20 row tiles
DC = D // 128             # 4 contraction chunks
SCALE = 1.0 / float(np.sqrt(D))

for _p in ("/opt/trn_rl_repo",):
    if os.path.isdir(_p) and _p not in sys.path:
        sys.path.insert(0, _p)

LAST_EXEC_NS = None
LAST_TRACE = None
_PROG_CACHE = {}


# ----------------------------------------------------------------- fallback --
def _np_ln(x, g, b):
    mu = x.mean(axis=-1, keepdims=True, dtype=np.float32)
    xc = x - mu
    var = np.mean(xc * xc, axis=-1, keepdims=True, dtype=np.float32)
    return xc / np.sqrt(var + LN_EPS) * g + b


def _np_kernel(imgs, caps, img_lens, cap_lens, Wq, bq, Wk, bk, Wv, bv, Wo, bo,
               g1, b1, g2, b2, g3, b3, g4, b4):
    bi, r, d = imgs.shape
    bc, w, _ = caps.shape
    img_valid = np.arange(r)[None, :] < img_lens[:, None]
    cap_valid = np.arange(w)[None, :] < cap_lens[:, None]
    imgs_m = (imgs * img_valid[..., None]).astype(np.float32)
    caps_m = (caps * cap_valid[..., None]).astype(np.float32)
    q = (_np_ln(caps_m, g1, b1).reshape(bc * w, d) @ Wq.T + bq).astype(np.float32)
    lni = _np_ln(imgs_m, g2, b2).reshape(bi * r, d)
    k = (lni @ Wk.T + bk).reshape(bi, r, d).astype(np.float32)
    lni3 = _np_ln(imgs_m, g3, b3).reshape(bi * r, d)
    v = ((lni3 @ Wv.T + bv) * img_valid.reshape(bi * r, 1)).reshape(bi, r, d)
    sims = (q @ k.reshape(bi * r, d).T) * np.float32(1.0 / np.sqrt(d))
    sims = sims.reshape(bc, w, bi, r)
    pm = cap_valid[:, :, None, None] & img_valid[None, None, :, :]
    sims = np.where(pm, sims, np.float32(-1e30))
    sims -= sims.max(axis=-1, keepdims=True)
    np.exp(sims, out=sims)
    sims /= sims.sum(axis=-1, keepdims=True)
    attn = np.where(pm, sims, np.float32(0.0))
    attn_b = np.ascontiguousarray(attn.transpose(2, 0, 1, 3)).reshape(bi, bc * w, r)
    ctx = np.matmul(attn_b, v.astype(np.float32))
    out = _np_ln(ctx, g4, b4).reshape(bi * bc * w, d) @ Wo.T + bo
    out = out.reshape(bi, bc * w, d).astype(np.float32)
    num = np.einsum('bnd,nd->bn', out, q, optimize=True)
    den = np.sqrt((out * out).sum(axis=-1)) + np.float32(EPS)
    s = (num / den).reshape(bi, bc, w)
    s = np.where(cap_valid[None, :, :], s, np.float32(-1.0))
    return s.astype(np.float32)


# ------------------------------------------------------------ device program --
def _build_program():
    import os as _os
    FL = set((_os.environ.get("K_FLAGS") or "perimgat").split(","))
    import concourse.bass as bass
    import concourse.tile as tile
    from concourse import bacc, mybir
    from concourse.masks import make_identity
    from contextlib import ExitStack

    dt = mybir.dt
    f32, bf16 = dt.float32, dt.bfloat16
    AF = mybir.ActivationFunctionType
    OP = mybir.AluOpType
    AX = mybir.AxisListType

    nc = bacc.Bacc()
    caps_d = nc.dram_tensor("caps", (NCW, D), bf16, kind="ExternalInput")
    capsT_d = nc.dram_tensor("capsT", (D, NCW), bf16, kind="ExternalInput")
    imgs_d = nc.dram_tensor("imgs", (IR, D), bf16, kind="ExternalInput")
    wq_d = nc.dram_tensor("wq", (D, D), bf16, kind="ExternalInput")
    wk_d = nc.dram_tensor("wk", (D, D), bf16, kind="ExternalInput")
    wv_d = nc.dram_tensor("wv", (D, D), bf16, kind="ExternalInput")
    wo_d = nc.dram_tensor("wo", (D, D), bf16, kind="ExternalInput")
    wg1_d = nc.dram_tensor("wg1n", (D,), bf16, kind="ExternalInput")
    bvec_d = nc.dram_tensor("bvec", (128, 16), f32, kind="ExternalInput")
    m01_d = nc.dram_tensor("mask01", (IR,), bf16, kind="ExternalInput")
    capm_d = nc.dram_tensor("capm2", (NCW, 2), f32, kind="ExternalInput")
    out_d = nc.dram_tensor("out", (BI_S, NCW), f32, kind="ExternalOutput")

    irows = [(0, 128), (128, 128), (256, 32)]

    with ExitStack() as ctx:
        tc = ctx.enter_context(tile.TileContext(nc))
        big = ctx.enter_context(tc.tile_pool(name="big", bufs=1))
        wrk = ctx.enter_context(tc.tile_pool(name="wrk", bufs=3))
        mvp = ctx.enter_context(tc.tile_pool(name="mvp", bufs=6))
        apl = ctx.enter_context(tc.tile_pool(name="apl", bufs=3))
        atp = ctx.enter_context(tc.tile_pool(name="atp", bufs=8))
        scr = ctx.enter_context(tc.tile_pool(name="scr", bufs=6))
        psb = ctx.enter_context(tc.tile_pool(name="psb", bufs=6, space="PSUM"))
        pst = ctx.enter_context(tc.tile_pool(name="pst", bufs=2, space="PSUM"))

        ident = big.tile([128, 128], bf16, tag="ident")
        make_identity(nc, ident)
        epsi = big.tile([128, 1], f32, tag="epsi")
        nc.vector.memset(epsi, float(LN_EPS))
        epsc = big.tile([128, 1], f32, tag="epsc")
        nc.vector.memset(epsc, float(LN_EPS * D))

        wsb = {}
        for nm, d_ in (("wq", wq_d), ("wk", wk_d), ("wv", wv_d), ("wo", wo_d)):
            t_ = big.tile([128, DC, D], bf16, tag=nm)
            nc.sync.dma_start(out=t_, in_=d_[:, :].rearrange("(k p) d -> p k d", p=128))
            wsb[nm] = t_
        wg1row = big.tile([1, D], bf16, tag="wg1row")
        nc.sync.dma_start(out=wg1row, in_=wg1_d[:])
        bvec = big.tile([128, 16], f32, tag="bvec")
        nc.sync.dma_start(out=bvec, in_=bvec_d[:, :])
        mask = big.tile([128, BI_S, R], bf16, tag="mask")
        m01_ap = m01_d[:]
        m01_b = bass.AP(tensor=m01_ap.tensor, offset=m01_ap.offset,
                        ap=[[0, 128]] + list(m01_ap.ap))
        nc.gpsimd.dma_start(out=mask, in_=m01_b)
        capm = big.tile([128, NT, 2], f32, tag="capm")
        nc.sync.dma_start(out=capm, in_=capm_d[:, :].rearrange("(t p) c -> p t c", p=128))
        capsin = big.tile([128, NT, D], bf16, tag="capsin")
        nc.sync.dma_start(out=capsin,
                          in_=caps_d[:, :].rearrange("(t p) d -> p t d", p=128))
        capsT = big.tile([128, DC, NCW], bf16, tag="capsT")
        nc.sync.dma_start(out=capsT,
                          in_=capsT_d[:, :].rearrange("(k p) n -> p k n", p=128))
        imgsin = big.tile([128, 3, D], bf16, tag="imgsin")
        for ti, (r0, p) in enumerate(irows):
            nc.sync.dma_start(out=imgsin[:p, ti, :], in_=imgs_d[r0:r0 + p, :])

        # ---------------- phase A: image side (per-core slice, 288 rows) ----
        lniT = big.tile([128, DC, IR], bf16, tag="lniT")
        for ti, (r0, p) in enumerate(irows):
            x = imgsin[:, ti, :]
            st = mvp.tile([128, 6], f32, tag="st")
            nc.vector.bn_stats(st[:p], x[:p])
            ag = mvp.tile([128, 2], f32, tag="ag")
            nc.vector.bn_aggr(ag[:p], st[:p])
            sg = mvp.tile([128, 1], f32, tag="sg")
            nc.scalar.activation(sg[:p], ag[:p, 1:2], AF.Sqrt, bias=epsi[:p])
            iv = mvp.tile([128, 1], f32, tag="iv")
            nc.vector.reciprocal(iv[:p], sg[:p])
            ln = wrk.tile([128, D], bf16, tag="ln")
            nc.vector.tensor_scalar(out=ln[:p], in0=x[:p],
                                    scalar1=ag[:p, 0:1], scalar2=iv[:p],
                                    op0=OP.subtract, op1=OP.mult)
            for j in range(DC):
                pt = pst.tile([128, 128], bf16, tag="tr")
                nc.tensor.transpose(pt[:, :p], ln[:p, 128 * j:128 * j + 128],
                                    ident[:p, :p])
                nc.scalar.copy(out=lniT[:, j, r0:r0 + p], in_=pt[:, :p])

        # K^T (+bk') -- d on partitions
        kT = big.tile([128, DC, IR], bf16, tag="kT")
        for j in range(DC):
            ps = psb.tile([128, IR], f32, tag="ps")
            for k in range(DC):
                nc.tensor.matmul(ps, lhsT=wsb["wk"][:, k, 128 * j:128 * j + 128],
                                 rhs=lniT[:, k, :], start=(k == 0), stop=(k == DC - 1))
            nc.scalar.activation(kT[:, j, :], ps, AF.Identity,
                                 bias=bvec[:, 4 + j:5 + j])

        # V natural -> row-center -> V'^T (+bvc)
        vpT = big.tile([128, DC, IR], bf16, tag="vpT")
        for ti, (r0, p) in enumerate(irows):
            ps = psb.tile([128, D], f32, tag="ps")
            for k in range(DC):
                nc.tensor.matmul(ps[:p], lhsT=lniT[:, k, r0:r0 + p],
                                 rhs=wsb["wv"][:, k, :], start=(k == 0), stop=(k == DC - 1))
            ms = mvp.tile([128, 1], f32, tag="ms")
            nc.vector.reduce_sum(ms[:p], ps[:p], axis=AX.X)
            mu = mvp.tile([128, 1], f32, tag="mu")
            nc.scalar.mul(mu[:p], ms[:p], 1.0 / D)
            vb = wrk.tile([128, D], bf16, tag="vb")
            nc.vector.tensor_scalar_sub(out=vb[:p], in0=ps[:p], scalar1=mu[:p])
            for j in range(DC):
                pt = pst.tile([128, 128], bf16, tag="tr")
                nc.tensor.transpose(pt[:, :p], vb[:p, 128 * j:128 * j + 128],
                                    ident[:p, :p])
                nc.scalar.activation(vpT[:, j, r0:r0 + p], pt[:, :p], AF.Identity,
                                     bias=bvec[:, 12 + j:13 + j])

        # P^T = Wo_g4 @ V'^T + pc
        pT = big.tile([128, DC, IR], bf16, tag="pT")
        for j in range(DC):
            ps = psb.tile([128, IR], f32, tag="ps")
            for k in range(DC):
                nc.tensor.matmul(ps, lhsT=wsb["wo"][:, k, 128 * j:128 * j + 128],
                                 rhs=vpT[:, k, :], start=(k == 0), stop=(k == DC - 1))
            nc.scalar.activation(pT[:, j, :], ps, AF.Identity,
                                 bias=bvec[:, 8 + j:9 + j])

        # M Gram per image (duplicated at partition bases 0 and 64)
        sm = big.tile([128, BI_S, R], bf16, tag="sm")
        for i in range(BI_S):
            gps = pst.tile([36, 36], f32, tag="tr")
            for k in range(DC):
                nc.tensor.matmul(gps, lhsT=pT[:, k, R * i:R * i + R],
                                 rhs=pT[:, k, R * i:R * i + R],
                                 start=(k == 0), stop=(k == DC - 1))
            nc.scalar.copy(sm[0:R, i, :], gps)
            nc.scalar.copy(sm[64:64 + R, i, :], gps)

        # ---------------- phase B: caption stats + centered Q^T projection --
        qT = big.tile([128, DC, NCW], bf16, tag="qT")
        isv = big.tile([128, NT], f32, tag="isv")
        muRow = big.tile([1, NCW], bf16, tag="muRow")
        for t in range(NT):
            x = capsin[:, t, :]
            st = mvp.tile([128, 6], f32, tag="st")
            nc.vector.bn_stats(st, x)
            ag = mvp.tile([128, 2], f32, tag="ag")
            nc.vector.bn_aggr(ag, st)
            sg = mvp.tile([128, 1], f32, tag="sg")
            nc.scalar.activation(sg, ag[:, 1:2], AF.Sqrt, bias=epsc, scale=float(D))
            nc.vector.reciprocal(isv[:, t:t + 1], sg)
            if "nomu" not in FL:
                mb_ = mvp.tile([128, 1], bf16, tag="mb")
                nc.vector.tensor_copy(mb_, ag[:, 0:1])
                pt = pst.tile([128, 128], bf16, tag="tr")
                nc.tensor.transpose(pt[0:1, :], mb_, ident)
                nc.vector.tensor_copy(muRow[0:1, 128 * t:128 * t + 128], pt[0:1, :])
            if t % 4 == 3:
                c = t // 4
                for j in range(DC):
                    ps = psb.tile([128, 512], f32, tag="ps")
                    for k in range(DC):
                        nc.tensor.matmul(ps, lhsT=wsb["wq"][:, k, 128 * j:128 * j + 128],
                                         rhs=capsT[:, k, 512 * c:512 * c + 512],
                                         start=(k == 0),
                                         stop=("nomu" in FL and k == DC - 1))
                    if "nomu" not in FL:
                        nc.tensor.matmul(ps, lhsT=wg1row[0:1, 128 * j:128 * j + 128],
                                         rhs=muRow[0:1, 512 * c:512 * c + 512],
                                         start=False, stop=True)
                    nc.scalar.activation(qT[:, j, 512 * c:512 * c + 512], ps,
                                         AF.Identity, bias=bvec[:, j:j + 1])

        # ---------------- phase C: per caption-word tile ---------------------
        nTall = big.tile([128, NT, BI_S], f32, tag="nTall")
        nMall = big.tile([128, NT, BI_S], f32, tag="nMall")
        for t in range(NT):
            cw = slice(128 * t, 128 * t + 128)
            psK = psb.tile([128, BI_S, R], f32, tag="ps")
            psT = psb.tile([128, BI_S, R], f32, tag="ps")
            for k in range(DC):
                nc.tensor.matmul(psK, lhsT=qT[:, k, cw], rhs=kT[:, k, :],
                                 start=(k == 0), stop=(k == DC - 1))
            for k in range(DC):
                nc.tensor.matmul(psT, lhsT=qT[:, k, cw], rhs=pT[:, k, :],
                                 start=(k == 0), stop=(k == DC - 1))
            A = apl.tile([128, BI_S, 64], bf16, tag="A")
            if "fullmemset" in FL:
                nc.gpsimd.memset(A, 0.0)
            else:
                nc.gpsimd.memset(A[:, :, R:64], 0.0)
            if "expconst" in FL:
                nc.scalar.activation(A[:, :, 0:R], psK, AF.Exp, scale=float(SCALE))
            else:
                nc.scalar.activation(A[:, :, 0:R], psK, AF.Exp, scale=isv[:, t:t + 1])
            nc.vector.tensor_mul(A[:, :, 0:R], A[:, :, 0:R], mask)

            ats = []
            if "perimgat" in FL:
                for i in range(BI_S):
                    pt = pst.tile([128, 128], bf16, tag="tr")
                    nc.tensor.transpose(pt[:R, :], A[:, i, 0:R], ident)
                    at = atp.tile([64, 128], bf16, tag="at")
                    nc.vector.tensor_copy(at[:R, :], pt[:R, :])
                    ats.append(at)
                psB = psb.tile([128, BI_S, R], f32, tag="ps")
                for i in range(BI_S):
                    nc.tensor.matmul(psB[:, i, :], lhsT=ats[i][:R, :],
                                     rhs=sm[0:R, i, :], start=True, stop=True)
            else:
                # image pairs packed at partition bases 0 / 64
                for h in range(4):
                    pt = pst.tile([128, 128], bf16, tag="tr")
                    nc.tensor.transpose(pt[0:R, :], A[:, 2 * h, 0:R], ident)
                    nc.tensor.transpose(pt[64:64 + R, :], A[:, 2 * h + 1, 0:R],
                                        ident)
                    at = atp.tile([128, 128], bf16, tag="at")
                    nc.vector.tensor_copy(at, pt)
                    ats.append(at)
                psB = psb.tile([128, BI_S, R], f32, tag="ps")
                for i in range(BI_S):
                    base = 0 if i % 2 == 0 else 64
                    nc.tensor.matmul(psB[:, i, :],
                                     lhsT=ats[i // 2][base:base + R, :],
                                     rhs=sm[base:base + R, i, :], start=True, stop=True)

            scT = scr.tile([128, BI_S, R], bf16, tag="scT")
            nc.vector.tensor_mul(scT, A[:, :, 0:R], psT)
            nc.vector.reduce_sum(nTall[:, t, :], scT, axis=AX.X)
            scM = scr.tile([128, BI_S, R], bf16, tag="scM")
            nc.vector.tensor_mul(scM, A[:, :, 0:R], psB)
            nc.vector.reduce_sum(nMall[:, t, :], scM, axis=AX.X)

        # ---------------- batched epilogue ----------------------------------
        def bcast8(ap2d):
            return bass.AP(tensor=ap2d.tensor, offset=ap2d.offset,
                           ap=list(ap2d.ap) + [[0, BI_S]])

        sqM = big.tile([128, NT, BI_S], f32, tag="sqM")
        nc.scalar.activation(sqM, nMall, AF.Sqrt)
        nc.gpsimd.tensor_scalar_add(sqM, sqM, 1e-12)
        rr = big.tile([128, NT, BI_S], f32, tag="rr")
        nc.vector.reciprocal(rr, sqM)
        nTs = big.tile([128, NT, BI_S], f32, tag="nTs")
        nc.gpsimd.tensor_mul(nTs, nTall, bcast8(isv[:, :]))
        s0 = big.tile([128, NT, BI_S], f32, tag="s0")
        nc.gpsimd.tensor_mul(s0, nTs, rr)
        s1 = big.tile([128, NT, BI_S], f32, tag="s1")
        nc.gpsimd.tensor_mul(s1, s0, bcast8(capm[:, :, 0]))
        sf = big.tile([128, BI_S, NT], f32, tag="sf")
        nc.gpsimd.tensor_add(sf[:, :, :].rearrange("p i t -> p t i"), s1,
                             bcast8(capm[:, :, 1]))
        if "tiledma" in FL:
            for t in range(NT):
                nc.sync.dma_start(
                    out=out_d[:, 128 * t:128 * t + 128].rearrange("i p -> p i"),
                    in_=sf[:, :, t])
        else:
            nc.sync.dma_start(
                out=out_d[:, :].rearrange("i (t p) -> p (i t)", p=128), in_=sf)

    nc.finalize()
    return nc


def _get_program():
    if "nc" not in _PROG_CACHE:
        _PROG_CACHE["nc"] = _build_program()
    return _PROG_CACHE["nc"]


# ------------------------------------------------------------------- driver --
def kernel(imgs, caps, img_lens, cap_lens,
           Wq, bq, Wk, bk, Wv, bv, Wo, bo,
           g1, b1, g2, b2, g3, b3, g4, b4):
    global LAST_EXEC_NS, LAST_TRACE
    args = dict(imgs=imgs, caps=caps, img_lens=img_lens, cap_lens=cap_lens,
                Wq=Wq, bq=bq, Wk=Wk, bk=bk, Wv=Wv, bv=bv, Wo=Wo, bo=bo,
                g1=g1, b1=b1, g2=g2, b2=b2, g3=g3, b3=b3, g4=g4, b4=b4)
    args = {k: np.asarray(v, np.float32) if np.asarray(v).dtype != np.int32
            else np.asarray(v) for k, v in args.items()}
    imgs, caps = args["imgs"], args["caps"]
    img_lens, cap_lens = np.asarray(img_lens, np.int32), np.asarray(cap_lens, np.int32)
    c0 = args["Wo"] @ args["b4"] + args["bo"]
    qb0 = args["Wq"] @ args["b1"] + args["bq"]
    if ((imgs.shape, caps.shape) != ((Bi, R, D), (Bc, W, D))
            or np.abs(c0).max() != 0 or np.abs(qb0).max() != 0):
        return _np_kernel(**args)
    try:
        return _device_kernel(args, img_lens, cap_lens)
    except Exception:
        import traceback
        traceback.print_exc()
        print("kernel: device path failed; falling back to numpy", file=sys.stderr)
        return _np_kernel(**args)


def _device_kernel(a, img_lens, cap_lens):
    global LAST_EXEC_NS, LAST_TRACE
    import ml_dtypes
    from concourse.bass_utils import run_bass_kernel_spmd

    bf = ml_dtypes.bfloat16
    img_valid = (np.arange(R)[None, :] < img_lens[:, None])
    cap_valid = (np.arange(W)[None, :] < cap_lens[:, None])
    imgs_m = (a["imgs"] * img_valid[..., None]).reshape(Bi * R, D)
    caps_m = (a["caps"] * cap_valid[..., None]).reshape(NCW, D)

    Wq_g = a["Wq"] * a["g1"][None, :]
    WqgT = np.ascontiguousarray(Wq_g.T).astype(bf)
    WkgT = np.ascontiguousarray((a["Wk"] * a["g2"][None, :]).T).astype(bf)
    WvgT = np.ascontiguousarray((a["Wv"] * a["g3"][None, :]).T).astype(bf)
    WogT = np.ascontiguousarray((a["Wo"] * a["g4"][None, :]).T).astype(bf)
    wg1n = np.ascontiguousarray(-Wq_g.sum(axis=1)).astype(bf)
    qb = a["Wq"] @ a["b1"] + a["bq"]
    bk_ = a["Wk"] @ a["b2"] + a["bk"]
    bv_ = a["Wv"] @ a["b3"] + a["bv"]
    bvc = (bv_ - bv_.mean()).astype(np.float32)
    pc = (a["Wo"] * a["g4"][None, :]) @ bvc
    bvec = np.stack([qb.reshape(DC, 128), bk_.reshape(DC, 128),
                     pc.reshape(DC, 128), bvc.reshape(DC, 128)],
                    axis=0).reshape(16, 128).T
    bvec = np.ascontiguousarray(bvec, dtype=np.float32)  # (128, 16)

    capm = cap_valid.reshape(NCW, 1).astype(np.float32)
    capm2 = np.ascontiguousarray(
        np.concatenate([capm * np.float32(np.sqrt(D)), capm - 1.0], axis=1))

    caps_bf = np.ascontiguousarray(caps_m).astype(bf)
    capsT_bf = np.ascontiguousarray(caps_m.T).astype(bf)
    in_maps = []
    for c in range(N_CORES):
        sl = slice(c * BI_S * R, (c + 1) * BI_S * R)
        in_maps.append({
            "caps": caps_bf,
            "capsT": capsT_bf,
            "imgs": np.ascontiguousarray(imgs_m[sl]).astype(bf),
            "wq": WqgT, "wk": WkgT, "wv": WvgT, "wo": WogT,
            "wg1n": wg1n,
            "bvec": bvec,
            "mask01": np.ascontiguousarray(
                img_valid[c * BI_S:(c + 1) * BI_S].reshape(IR)).astype(bf),
            "capm2": capm2,
        })

    nc = _get_program()
    trace = bool(os.environ.get("BASS_KTRACE"))
    kw = {}
    if trace:
        kw = dict(trace=True, tmpdir=os.environ.get("BASS_KTRACE_DIR") or None)
    res = run_bass_kernel_spmd(nc, in_maps, list(range(N_CORES)), **kw)
    if trace:
        LAST_EXEC_NS = res.exec_time_ns
        LAST_TRACE = res.profile_json
    out = np.concatenate(
        [r["out"].reshape(BI_S, Bc, W) for r in res.results], axis=0)
    return np.ascontiguousarray(out.astype(np.float32))


# revision 29
# speedup vs baseline: 11.4926x; 11.4926x over previous
"""Trainium2 Bass kernel for the fine-grained caption/image sparse-attention
similarity module.

Math (per image i, caption-word row x = (c,w)):
    q = LN(caps;g1,b1)@Wq^T + bq          -> folded: LNraw@WqgT + qb
    k = LN(imgs;g2,b2)@Wk^T + bk          -> folded: LNraw@WkgT + bk'
    v = LN(imgs;g3,b3)@Wv^T + bv          -> folded: LNraw@WvgT + bv'
    e[x,i,r]  = exp(q.k/sqrt(D)) * mask01[i,r]        (unnormalized attn)
    ssum      = sum_r e;  a = e/ssum                  (true attn)
    ctx       = a@V_i  (rank-36!)  ->  never materialized:
      V'_i = V_i - rowmean(V_i)   (row-centered -> LN centering is exact)
      P_i  = V'_i @ (Wo*g4)^T
      S_i  = V'_i V'_i^T / D,  M_i = P_i P_i^T        (36x36 Grams)
    g   = sqrt(e^T S_i e + LN_EPS*ssum^2)             ( = ssum*sqrt(var+eps) )
    num = e.(Q@P_i^T) ;  nM = e^T M_i e
    s   = num / (sqrt(nM) + EPS*g)                    (== cos-sim numerically)
    out[i, x] = s*capm + (capm-1)                     (invalid cap words -> -1)

Sharding: 8 images per core (replicated captions/weights), no collectives.
"""

import os
import sys

import numpy as np

EPS = 1e-8
LN_EPS = 1e-5

Bi, R, Bc, W, D = 64, 36, 64, 40, 512
N_CORES = 8
BI_S = Bi // N_CORES      # 8 images per core
IR = BI_S * R             # 288
NCW = Bc * W              # 2560 caption-word rows
NT = NCW // 128           # 20 row tiles
DC = D // 128             # 4 contraction chunks
SCALE = 1.0 / float(np.sqrt(D))

for _p in ("/opt/trn_rl_repo",):
    if os.path.isdir(_p) and _p not in sys.path:
        sys.path.insert(0, _p)

LAST_EXEC_NS = None
LAST_TRACE = None
_PROG_CACHE = {}


# ----------------------------------------------------------------- fallback --
def _np_ln(x, g, b):
    mu = x.mean(axis=-1, keepdims=True, dtype=np.float32)
    xc = x - mu
    var = np.mean(xc * xc, axis=-1, keepdims=True, dtype=np.float32)
    return xc / np.sqrt(var + LN_EPS) * g + b


def _np_kernel(imgs, caps, img_lens, cap_lens, Wq, bq, Wk, bk, Wv, bv, Wo, bo,
               g1, b1, g2, b2, g3, b3, g4, b4):
    bi, r, d = imgs.shape
    bc, w, _ = caps.shape
    img_valid = np.arange(r)[None, :] < img_lens[:, None]
    cap_valid = np.arange(w)[None, :] < cap_lens[:, None]
    imgs_m = (imgs * img_valid[..., None]).astype(np.float32)
    caps_m = (caps * cap_valid[..., None]).astype(np.float32)
    q = (_np_ln(caps_m, g1, b1).reshape(bc * w, d) @ Wq.T + bq).astype(np.float32)
    lni = _np_ln(imgs_m, g2, b2).reshape(bi * r, d)
    k = (lni @ Wk.T + bk).reshape(bi, r, d).astype(np.float32)
    lni3 = _np_ln(imgs_m, g3, b3).reshape(bi * r, d)
    v = ((lni3 @ Wv.T + bv) * img_valid.reshape(bi * r, 1)).reshape(bi, r, d)
    sims = (q @ k.reshape(bi * r, d).T) * np.float32(1.0 / np.sqrt(d))
    sims = sims.reshape(bc, w, bi, r)
    pm = cap_valid[:, :, None, None] & img_valid[None, None, :, :]
    sims = np.where(pm, sims, np.float32(-1e30))
    sims -= sims.max(axis=-1, keepdims=True)
    np.exp(sims, out=sims)
    sims /= sims.sum(axis=-1, keepdims=True)
    attn = np.where(pm, sims, np.float32(0.0))
    attn_b = np.ascontiguousarray(attn.transpose(2, 0, 1, 3)).reshape(bi, bc * w, r)
    ctx = np.matmul(attn_b, v.astype(np.float32))
    out = _np_ln(ctx, g4, b4).reshape(bi * bc * w, d) @ Wo.T + bo
    out = out.reshape(bi, bc * w, d).astype(np.float32)
    num = np.einsum('bnd,nd->bn', out, q, optimize=True)
    den = np.sqrt((out * out).sum(axis=-1)) + np.float32(EPS)
    s = (num / den).reshape(bi, bc, w)
    s = np.where(cap_valid[None, :, :], s, np.float32(-1.0))
    return s.astype(np.float32)


# ------------------------------------------------------------ device program --
def _build_program():
    import os as _os
    FL = set((_os.environ.get("K_FLAGS") or "perimgat").split(","))
    import concourse.bass as bass
    import concourse.tile as tile
    from concourse import bacc, mybir
    from concourse.masks import make_identity
    from contextlib import ExitStack

    dt = mybir.dt
    f32, bf16 = dt.float32, dt.bfloat16
    AF = mybir.ActivationFunctionType
    OP = mybir.AluOpType
    AX = mybir.AxisListType

    nc = bacc.Bacc()
    caps_d = nc.dram_tensor("caps", (NCW, D), bf16, kind="ExternalInput")
    capsT_d = nc.dram_tensor("capsT", (D, NCW), bf16, kind="ExternalInput")
    imgs_d = nc.dram_tensor("imgs", (IR, D), bf16, kind="ExternalInput")
    wq_d = nc.dram_tensor("wq", (D, D), bf16, kind="ExternalInput")
    wk_d = nc.dram_tensor("wk", (D, D), bf16, kind="ExternalInput")
    wv_d = nc.dram_tensor("wv", (D, D), bf16, kind="ExternalInput")
    wo_d = nc.dram_tensor("wo", (D, D), bf16, kind="ExternalInput")
    wg1_d = nc.dram_tensor("wg1n", (D,), bf16, kind="ExternalInput")
    bvec_d = nc.dram_tensor("bvec", (128, 16), f32, kind="ExternalInput")
    m01_d = nc.dram_tensor("mask01", (IR,), bf16, kind="ExternalInput")
    capm_d = nc.dram_tensor("capm2", (NCW, 2), f32, kind="ExternalInput")
    out_d = nc.dram_tensor("out", (BI_S, NCW), f32, kind="ExternalOutput")

    irows = [(0, 128), (128, 128), (256, 32)]

    with ExitStack() as ctx:
        tc = ctx.enter_context(tile.TileContext(nc))
        big = ctx.enter_context(tc.tile_pool(name="big", bufs=1))
        wrk = ctx.enter_context(tc.tile_pool(name="wrk", bufs=3))
        mvp = ctx.enter_context(tc.tile_pool(name="mvp", bufs=6))
        apl = ctx.enter_context(tc.tile_pool(name="apl", bufs=3))
        atp = ctx.enter_context(tc.tile_pool(name="atp", bufs=8))
        scr = ctx.enter_context(tc.tile_pool(name="scr", bufs=6))
        psb = ctx.enter_context(tc.tile_pool(name="psb", bufs=6, space="PSUM"))
        pst = ctx.enter_context(tc.tile_pool(name="pst", bufs=2, space="PSUM"))

        ident = big.tile([128, 128], bf16, tag="ident")
        make_identity(nc, ident)
        epsi = big.tile([128, 1], f32, tag="epsi")
        nc.vector.memset(epsi, float(LN_EPS))
        epsc = big.tile([128, 1], f32, tag="epsc")
        nc.vector.memset(epsc, float(LN_EPS * D))

        wsb = {}
        for nm, d_ in (("wq", wq_d), ("wk", wk_d), ("wv", wv_d), ("wo", wo_d)):
            t_ = big.tile([128, DC, D], bf16, tag=nm)
            nc.sync.dma_start(out=t_, in_=d_[:, :].rearrange("(k p) d -> p k d", p=128))
            wsb[nm] = t_
        wg1row = big.tile([1, D], bf16, tag="wg1row")
        nc.sync.dma_start(out=wg1row, in_=wg1_d[:])
        bvec = big.tile([128, 16], f32, tag="bvec")
        nc.sync.dma_start(out=bvec, in_=bvec_d[:, :])
        mask = big.tile([128, BI_S, R], bf16, tag="mask")
        m01_ap = m01_d[:]
        m01_b = bass.AP(tensor=m01_ap.tensor, offset=m01_ap.offset,
                        ap=[[0, 128]] + list(m01_ap.ap))
        nc.gpsimd.dma_start(out=mask, in_=m01_b)
        capm = big.tile([128, NT, 2], f32, tag="capm")
        nc.sync.dma_start(out=capm, in_=capm_d[:, :].rearrange("(t p) c -> p t c", p=128))
        capsin = big.tile([128, NT, D], bf16, tag="capsin")
        nc.sync.dma_start(out=capsin,
                          in_=caps_d[:, :].rearrange("(t p) d -> p t d", p=128))
        capsT = big.tile([128, DC, NCW], bf16, tag="capsT")
        nc.sync.dma_start(out=capsT,
                          in_=capsT_d[:, :].rearrange("(k p) n -> p k n", p=128))
        imgsin = big.tile([128, 3, D], bf16, tag="imgsin")
        for ti, (r0, p) in enumerate(irows):
            nc.sync.dma_start(out=imgsin[:p, ti, :], in_=imgs_d[r0:r0 + p, :])

        # ---------------- phase A: image side (per-core slice, 288 rows) ----
        lniT = big.tile([128, DC, IR], bf16, tag="lniT")
        for ti, (r0, p) in enumerate(irows):
            x = imgsin[:, ti, :]
            st = mvp.tile([128, 6], f32, tag="st")
            nc.vector.bn_stats(st[:p], x[:p])
            ag = mvp.tile([128, 2], f32, tag="ag")
            nc.vector.bn_aggr(ag[:p], st[:p])
            sg = mvp.tile([128, 1], f32, tag="sg")
            nc.scalar.activation(sg[:p], ag[:p, 1:2], AF.Sqrt, bias=epsi[:p])
            iv = mvp.tile([128, 1], f32, tag="iv")
            nc.vector.reciprocal(iv[:p], sg[:p])
            ln = wrk.tile([128, D], bf16, tag="ln")
            nc.vector.tensor_scalar(out=ln[:p], in0=x[:p],
                                    scalar1=ag[:p, 0:1], scalar2=iv[:p],
                                    op0=OP.subtract, op1=OP.mult)
            for j in range(DC):
                pt = pst.tile([128, 128], bf16, tag="tr")
                nc.tensor.transpose(pt[:, :p], ln[:p, 128 * j:128 * j + 128],
                                    ident[:p, :p])
                nc.scalar.copy(out=lniT[:, j, r0:r0 + p], in_=pt[:, :p])

        # K^T (+bk') -- d on partitions
        kT = big.tile([128, DC, IR], bf16, tag="kT")
        for j in range(DC):
            ps = psb.tile([128, IR], f32, tag="ps")
            for k in range(DC):
                nc.tensor.matmul(ps, lhsT=wsb["wk"][:, k, 128 * j:128 * j + 128],
                                 rhs=lniT[:, k, :], start=(k == 0), stop=(k == DC - 1))
            nc.scalar.activation(kT[:, j, :], ps, AF.Identity,
                                 bias=bvec[:, 4 + j:5 + j])

        # V natural -> row-center -> V'^T (+bvc)
        vpT = big.tile([128, DC, IR], bf16, tag="vpT")
        for ti, (r0, p) in enumerate(irows):
            ps = psb.tile([128, D], f32, tag="ps")
            for k in range(DC):
                nc.tensor.matmul(ps[:p], lhsT=lniT[:, k, r0:r0 + p],
                                 rhs=wsb["wv"][:, k, :], start=(k == 0), stop=(k == DC - 1))
            ms = mvp.tile([128, 1], f32, tag="ms")
            nc.vector.reduce_sum(ms[:p], ps[:p], axis=AX.X)
            mu = mvp.tile([128, 1], f32, tag="mu")
            nc.scalar.mul(mu[:p], ms[:p], 1.0 / D)
            vb = wrk.tile([128, D], bf16, tag="vb")
            nc.vector.tensor_scalar_sub(out=vb[:p], in0=ps[:p], scalar1=mu[:p])
            for j in range(DC):
                pt = pst.tile([128, 128], bf16, tag="tr")
                nc.tensor.transpose(pt[:, :p], vb[:p, 128 * j:128 * j + 128],
                                    ident[:p, :p])
                nc.scalar.activation(vpT[:, j, r0:r0 + p], pt[:, :p], AF.Identity,
                                     bias=bvec[:, 12 + j:13 + j])

        # P^T = Wo_g4 @ V'^T + pc
        pT = big.tile([128, DC, IR], bf16, tag="pT")
        for j in range(DC):
            ps = psb.tile([128, IR], f32, tag="ps")
            for k in range(DC):
                nc.tensor.matmul(ps, lhsT=wsb["wo"][:, k, 128 * j:128 * j + 128],
                                 rhs=vpT[:, k, :], start=(k == 0), stop=(k == DC - 1))
            nc.scalar.activation(pT[:, j, :], ps, AF.Identity,
                                 bias=bvec[:, 8 + j:9 + j])
            nc.vector.tensor_mul(pT[:, j, :], pT[:, j, :],
                                 mask[:, :, :].rearrange("p a b -> p (a b)"))

        # M Gram per image (duplicated at partition bases 0 and 64)
        sm = big.tile([128, BI_S, R], bf16, tag="sm")
        for i in range(BI_S):
            gps = pst.tile([36, 36], f32, tag="tr")
            for k in range(DC):
                nc.tensor.matmul(gps, lhsT=pT[:, k, R * i:R * i + R],
                                 rhs=pT[:, k, R * i:R * i + R],
                                 start=(k == 0), stop=(k == DC - 1))
            nc.scalar.copy(sm[0:R, i, :], gps)
            nc.scalar.copy(sm[64:64 + R, i, :], gps)

        # ---------------- phase B: caption stats + centered Q^T projection --
        qT = big.tile([128, DC, NCW], bf16, tag="qT")
        isv = big.tile([128, NT], f32, tag="isv")
        muRow = big.tile([1, NCW], bf16, tag="muRow")
        for t in range(NT):
            x = capsin[:, t, :]
            st = mvp.tile([128, 6], f32, tag="st")
            nc.vector.bn_stats(st, x)
            ag = mvp.tile([128, 2], f32, tag="ag")
            nc.vector.bn_aggr(ag, st)
            sg = mvp.tile([128, 1], f32, tag="sg")
            nc.scalar.activation(sg, ag[:, 1:2], AF.Sqrt, bias=epsc, scale=float(D))
            nc.vector.reciprocal(isv[:, t:t + 1], sg)
            if "nomu" not in FL:
                mb_ = mvp.tile([128, 1], bf16, tag="mb")
                nc.vector.tensor_copy(mb_, ag[:, 0:1])
                pt = pst.tile([128, 128], bf16, tag="tr")
                nc.tensor.transpose(pt[0:1, :], mb_, ident)
                nc.vector.tensor_copy(muRow[0:1, 128 * t:128 * t + 128], pt[0:1, :])
            if t % 4 == 3:
                c = t // 4
                for j in range(DC):
                    ps = psb.tile([128, 512], f32, tag="ps")
                    for k in range(DC):
                        nc.tensor.matmul(ps, lhsT=wsb["wq"][:, k, 128 * j:128 * j + 128],
                                         rhs=capsT[:, k, 512 * c:512 * c + 512],
                                         start=(k == 0),
                                         stop=("nomu" in FL and k == DC - 1))
                    if "nomu" not in FL:
                        nc.tensor.matmul(ps, lhsT=wg1row[0:1, 128 * j:128 * j + 128],
                                         rhs=muRow[0:1, 512 * c:512 * c + 512],
                                         start=False, stop=True)
                    nc.scalar.activation(qT[:, j, 512 * c:512 * c + 512], ps,
                                         AF.Identity, bias=bvec[:, j:j + 1])

        # ---------------- phase C: per caption-word tile ---------------------
        nTall = big.tile([128, NT, BI_S], f32, tag="nTall")
        nMall = big.tile([128, NT, BI_S], f32, tag="nMall")
        for t in range(NT):
            cw = slice(128 * t, 128 * t + 128)
            psK = psb.tile([128, BI_S, R], f32, tag="ps")
            psT = psb.tile([128, BI_S, R], f32, tag="ps")
            for k in range(DC):
                nc.tensor.matmul(psK, lhsT=qT[:, k, cw], rhs=kT[:, k, :],
                                 start=(k == 0), stop=(k == DC - 1))
            for k in range(DC):
                nc.tensor.matmul(psT, lhsT=qT[:, k, cw], rhs=pT[:, k, :],
                                 start=(k == 0), stop=(k == DC - 1))
            A = apl.tile([128, BI_S, R], bf16, tag="A")
            nc.scalar.activation(A, psK, AF.Exp, scale=isv[:, t:t + 1])

            ats = []
            for i in range(BI_S):
                pt = pst.tile([128, 128], bf16, tag="tr")
                nc.tensor.transpose(pt[:R, :], A[:, i, :], ident)
                at = atp.tile([64, 128], bf16, tag="at")
                if i % 2 == 0:
                    nc.vector.tensor_copy(at[:R, :], pt[:R, :])
                else:
                    nc.scalar.copy(at[:R, :], pt[:R, :])
                ats.append(at)
            psB = psb.tile([128, BI_S, R], f32, tag="ps")
            for i in range(BI_S):
                nc.tensor.matmul(psB[:, i, :], lhsT=ats[i][:R, :],
                                 rhs=sm[0:R, i, :], start=True, stop=True)

            scT = scr.tile([128, BI_S, R], bf16, tag="scT")
            nc.vector.tensor_mul(scT, A, psT)
            nc.vector.reduce_sum(nTall[:, t, :], scT, axis=AX.X)
            scM = scr.tile([128, BI_S, R], bf16, tag="scM")
            nc.vector.tensor_mul(scM, A, psB)
            nc.vector.reduce_sum(nMall[:, t, :], scM, axis=AX.X)

        # ---------------- batched epilogue ----------------------------------
        def bcast8(ap2d):
            return bass.AP(tensor=ap2d.tensor, offset=ap2d.offset,
                           ap=list(ap2d.ap) + [[0, BI_S]])

        sqM = big.tile([128, NT, BI_S], f32, tag="sqM")
        nc.scalar.activation(sqM, nMall, AF.Sqrt)
        nc.gpsimd.tensor_scalar_add(sqM, sqM, 1e-12)
        rr = big.tile([128, NT, BI_S], f32, tag="rr")
        nc.vector.reciprocal(rr, sqM)
        nTs = big.tile([128, NT, BI_S], f32, tag="nTs")
        nc.gpsimd.tensor_mul(nTs, nTall, bcast8(isv[:, :]))
        s0 = big.tile([128, NT, BI_S], f32, tag="s0")
        nc.gpsimd.tensor_mul(s0, nTs, rr)
        s1 = big.tile([128, NT, BI_S], f32, tag="s1")
        nc.gpsimd.tensor_mul(s1, s0, bcast8(capm[:, :, 0]))
        sf = big.tile([128, BI_S, NT], f32, tag="sf")
        nc.gpsimd.tensor_add(sf[:, :, :].rearrange("p i t -> p t i"), s1,
                             bcast8(capm[:, :, 1]))
        if "tiledma" in FL:
            for t in range(NT):
                nc.sync.dma_start(
                    out=out_d[:, 128 * t:128 * t + 128].rearrange("i p -> p i"),
                    in_=sf[:, :, t])
        else:
            nc.sync.dma_start(
                out=out_d[:, :].rearrange("i (t p) -> p (i t)", p=128), in_=sf)

    nc.finalize()
    return nc


def _get_program():
    if "nc" not in _PROG_CACHE:
        _PROG_CACHE["nc"] = _build_program()
    return _PROG_CACHE["nc"]


# ------------------------------------------------------------------- driver --
def kernel(imgs, caps, img_lens, cap_lens,
           Wq, bq, Wk, bk, Wv, bv, Wo, bo,
           g1, b1, g2, b2, g3, b3, g4, b4):
    global LAST_EXEC_NS, LAST_TRACE
    args = dict(imgs=imgs, caps=caps, img_lens=img_lens, cap_lens=cap_lens,
                Wq=Wq, bq=bq, Wk=Wk, bk=bk, Wv=Wv, bv=bv, Wo=Wo, bo=bo,
                g1=g1, b1=b1, g2=g2, b2=b2, g3=g3, b3=b3, g4=g4, b4=b4)
    args = {k: np.asarray(v, np.float32) if np.asarray(v).dtype != np.int32
            else np.asarray(v) for k, v in args.items()}
    imgs, caps = args["imgs"], args["caps"]
    img_lens, cap_lens = np.asarray(img_lens, np.int32), np.asarray(cap_lens, np.int32)
    c0 = args["Wo"] @ args["b4"] + args["bo"]
    qb0 = args["Wq"] @ args["b1"] + args["bq"]
    if ((imgs.shape, caps.shape) != ((Bi, R, D), (Bc, W, D))
            or np.abs(c0).max() != 0 or np.abs(qb0).max() != 0):
        return _np_kernel(**args)
    try:
        return _device_kernel(args, img_lens, cap_lens)
    except Exception:
        import traceback
        traceback.print_exc()
        print("kernel: device path failed; falling back to numpy", file=sys.stderr)
        return _np_kernel(**args)


def _device_kernel(a, img_lens, cap_lens):
    global LAST_EXEC_NS, LAST_TRACE
    import ml_dtypes
    from concourse.bass_utils import run_bass_kernel_spmd

    bf = ml_dtypes.bfloat16
    img_valid = (np.arange(R)[None, :] < img_lens[:, None])
    cap_valid = (np.arange(W)[None, :] < cap_lens[:, None])
    imgs_m = (a["imgs"] * img_valid[..., None]).reshape(Bi * R, D)
    caps_m = (a["caps"] * cap_valid[..., None]).reshape(NCW, D)

    Wq_g = a["Wq"] * a["g1"][None, :]
    WqgT = np.ascontiguousarray(Wq_g.T).astype(bf)
    WkgT = np.ascontiguousarray((a["Wk"] * a["g2"][None, :]).T).astype(bf)
    WvgT = np.ascontiguousarray((a["Wv"] * a["g3"][None, :]).T).astype(bf)
    WogT = np.ascontiguousarray((a["Wo"] * a["g4"][None, :]).T).astype(bf)
    wg1n = np.ascontiguousarray(-Wq_g.sum(axis=1)).astype(bf)
    qb = a["Wq"] @ a["b1"] + a["bq"]
    bk_ = a["Wk"] @ a["b2"] + a["bk"]
    bv_ = a["Wv"] @ a["b3"] + a["bv"]
    bvc = (bv_ - bv_.mean()).astype(np.float32)
    pc = (a["Wo"] * a["g4"][None, :]) @ bvc
    bvec = np.stack([qb.reshape(DC, 128), bk_.reshape(DC, 128),
                     pc.reshape(DC, 128), bvc.reshape(DC, 128)],
                    axis=0).reshape(16, 128).T
    bvec = np.ascontiguousarray(bvec, dtype=np.float32)  # (128, 16)

    capm = cap_valid.reshape(NCW, 1).astype(np.float32)
    capm2 = np.ascontiguousarray(
        np.concatenate([capm * np.float32(np.sqrt(D)), capm - 1.0], axis=1))

    caps_bf = np.ascontiguousarray(caps_m).astype(bf)
    capsT_bf = np.ascontiguousarray(caps_m.T).astype(bf)
    in_maps = []
    for c in range(N_CORES):
        sl = slice(c * BI_S * R, (c + 1) * BI_S * R)
        in_maps.append({
            "caps": caps_bf,
            "capsT": capsT_bf,
            "imgs": np.ascontiguousarray(imgs_m[sl]).astype(bf),
            "wq": WqgT, "wk": WkgT, "wv": WvgT, "wo": WogT,
            "wg1n": wg1n,
            "bvec": bvec,
            "mask01": np.ascontiguousarray(
                img_valid[c * BI_S:(c + 1) * BI_S].reshape(IR)).astype(bf),
            "capm2": capm2,
        })

    nc = _get_program()
    trace = bool(os.environ.get("BASS_KTRACE"))
    kw = {}
    if trace:
        kw = dict(trace=True, tmpdir=os.environ.get("BASS_KTRACE_DIR") or None)
    res = run_bass_kernel_spmd(nc, in_maps, list(range(N_CORES)), **kw)
    if trace:
        LAST_EXEC_NS = res.exec_time_ns
        LAST_TRACE = res.profile_json
    out = np.concatenate(
        [r["out"].reshape(BI_S, Bc, W) for r in res.results], axis=0)
    return np.ascontiguousarray(out.astype(np.float32))


# revision 30
# speedup vs baseline: 22.7213x; 1.9770x over previous
"""Trainium2 Bass kernel for the fine-grained caption/image sparse-attention
similarity module.

Math (per image i, caption-word row x = (c,w)):
    q = LN(caps;g1,b1)@Wq^T + bq          -> folded: LNraw@WqgT + qb
    k = LN(imgs;g2,b2)@Wk^T + bk          -> folded: LNraw@WkgT + bk'
    v = LN(imgs;g3,b3)@Wv^T + bv          -> folded: LNraw@WvgT + bv'
    e[x,i,r]  = exp(q.k/sqrt(D)) * mask01[i,r]        (unnormalized attn)
    ssum      = sum_r e;  a = e/ssum                  (true attn)
    ctx       = a@V_i  (rank-36!)  ->  never materialized:
      V'_i = V_i - rowmean(V_i)   (row-centered -> LN centering is exact)
      P_i  = V'_i @ (Wo*g4)^T
      S_i  = V'_i V'_i^T / D,  M_i = P_i P_i^T        (36x36 Grams)
    g   = sqrt(e^T S_i e + LN_EPS*ssum^2)             ( = ssum*sqrt(var+eps) )
    num = e.(Q@P_i^T) ;  nM = e^T M_i e
    s   = num / (sqrt(nM) + EPS*g)                    (== cos-sim numerically)
    out[i, x] = s*capm + (capm-1)                     (invalid cap words -> -1)

Sharding: 8 images per core (replicated captions/weights), no collectives.
"""

import os
import sys

import numpy as np

EPS = 1e-8
LN_EPS = 1e-5

Bi, R, Bc, W, D = 64, 36, 64, 40, 512
N_CORES = 8
BI_S = Bi // N_CORES      # 8 images per core
IR = BI_S * R             # 288
NCW = Bc * W              # 2560 caption-word rows
NT = NCW // 128           # 20 row tiles
DC = D // 128             # 4 contraction chunks
SCALE = 1.0 / float(np.sqrt(D))

for _p in ("/opt/trn_rl_repo",):
    if os.path.isdir(_p) and _p not in sys.path:
        sys.path.insert(0, _p)

LAST_EXEC_NS = None
LAST_TRACE = None
_PROG_CACHE = {}


# ----------------------------------------------------------------- fallback --
def _np_ln(x, g, b):
    mu = x.mean(axis=-1, keepdims=True, dtype=np.float32)
    xc = x - mu
    var = np.mean(xc * xc, axis=-1, keepdims=True, dtype=np.float32)
    return xc / np.sqrt(var + LN_EPS) * g + b


def _np_kernel(imgs, caps, img_lens, cap_lens, Wq, bq, Wk, bk, Wv, bv, Wo, bo,
               g1, b1, g2, b2, g3, b3, g4, b4):
    bi, r, d = imgs.shape
    bc, w, _ = caps.shape
    img_valid = np.arange(r)[None, :] < img_lens[:, None]
    cap_valid = np.arange(w)[None, :] < cap_lens[:, None]
    imgs_m = (imgs * img_valid[..., None]).astype(np.float32)
    caps_m = (caps * cap_valid[..., None]).astype(np.float32)
    q = (_np_ln(caps_m, g1, b1).reshape(bc * w, d) @ Wq.T + bq).astype(np.float32)
    lni = _np_ln(imgs_m, g2, b2).reshape(bi * r, d)
    k = (lni @ Wk.T + bk).reshape(bi, r, d).astype(np.float32)
    lni3 = _np_ln(imgs_m, g3, b3).reshape(bi * r, d)
    v = ((lni3 @ Wv.T + bv) * img_valid.reshape(bi * r, 1)).reshape(bi, r, d)
    sims = (q @ k.reshape(bi * r, d).T) * np.float32(1.0 / np.sqrt(d))
    sims = sims.reshape(bc, w, bi, r)
    pm = cap_valid[:, :, None, None] & img_valid[None, None, :, :]
    sims = np.where(pm, sims, np.float32(-1e30))
    sims -= sims.max(axis=-1, keepdims=True)
    np.exp(sims, out=sims)
    sims /= sims.sum(axis=-1, keepdims=True)
    attn = np.where(pm, sims, np.float32(0.0))
    attn_b = np.ascontiguousarray(attn.transpose(2, 0, 1, 3)).reshape(bi, bc * w, r)
    ctx = np.matmul(attn_b, v.astype(np.float32))
    out = _np_ln(ctx, g4, b4).reshape(bi * bc * w, d) @ Wo.T + bo
    out = out.reshape(bi, bc * w, d).astype(np.float32)
    num = np.einsum('bnd,nd->bn', out, q, optimize=True)
    den = np.sqrt((out * out).sum(axis=-1)) + np.float32(EPS)
    s = (num / den).reshape(bi, bc, w)
    s = np.where(cap_valid[None, :, :], s, np.float32(-1.0))
    return s.astype(np.float32)


# ------------------------------------------------------------ device program --
def _build_program():
    import os as _os
    FL = set((_os.environ.get("K_FLAGS") or "perimgat").split(","))
    import concourse.bass as bass
    import concourse.tile as tile
    from concourse import bacc, mybir
    from concourse.masks import make_identity
    from contextlib import ExitStack

    dt = mybir.dt
    f32, bf16 = dt.float32, dt.bfloat16
    AF = mybir.ActivationFunctionType
    OP = mybir.AluOpType
    AX = mybir.AxisListType

    nc = bacc.Bacc()
    caps_d = nc.dram_tensor("caps", (NCW, D), bf16, kind="ExternalInput")
    capsT_d = nc.dram_tensor("capsT", (D, NCW), bf16, kind="ExternalInput")
    imgs_d = nc.dram_tensor("imgs", (IR, D), bf16, kind="ExternalInput")
    wq_d = nc.dram_tensor("wq", (D, D), bf16, kind="ExternalInput")
    wk_d = nc.dram_tensor("wk", (D, D), bf16, kind="ExternalInput")
    wv_d = nc.dram_tensor("wv", (D, D), bf16, kind="ExternalInput")
    wo_d = nc.dram_tensor("wo", (D, D), bf16, kind="ExternalInput")
    wg1_d = nc.dram_tensor("wg1n", (D,), bf16, kind="ExternalInput")
    bvec_d = nc.dram_tensor("bvec", (128, 16), f32, kind="ExternalInput")
    m01_d = nc.dram_tensor("mask01", (IR,), bf16, kind="ExternalInput")
    capm_d = nc.dram_tensor("capm2", (NCW, 2), f32, kind="ExternalInput")
    out_d = nc.dram_tensor("out", (BI_S, NCW), f32, kind="ExternalOutput")

    irows = [(0, 128), (128, 128), (256, 32)]

    with ExitStack() as ctx:
        tc = ctx.enter_context(tile.TileContext(nc))
        big = ctx.enter_context(tc.tile_pool(name="big", bufs=1))
        wrk = ctx.enter_context(tc.tile_pool(name="wrk", bufs=3))
        mvp = ctx.enter_context(tc.tile_pool(name="mvp", bufs=6))
        apl = ctx.enter_context(tc.tile_pool(name="apl", bufs=3))
        atp = ctx.enter_context(tc.tile_pool(name="atp", bufs=8))
        scr = ctx.enter_context(tc.tile_pool(name="scr", bufs=6))
        psb = ctx.enter_context(tc.tile_pool(name="psb", bufs=5, space="PSUM"))
        pst = ctx.enter_context(tc.tile_pool(name="pst", bufs=3, space="PSUM"))

        ident = big.tile([128, 128], bf16, tag="ident")
        make_identity(nc, ident)
        epsi = big.tile([128, 1], f32, tag="epsi")
        nc.vector.memset(epsi, float(LN_EPS))
        epsc = big.tile([128, 1], f32, tag="epsc")
        nc.vector.memset(epsc, float(LN_EPS * D))

        wsb = {}
        for nm, d_ in (("wq", wq_d), ("wk", wk_d), ("wv", wv_d), ("wo", wo_d)):
            t_ = big.tile([128, DC, D], bf16, tag=nm)
            nc.sync.dma_start(out=t_, in_=d_[:, :].rearrange("(k p) d -> p k d", p=128))
            wsb[nm] = t_
        wg1row = big.tile([1, D], bf16, tag="wg1row")
        nc.sync.dma_start(out=wg1row, in_=wg1_d[:])
        bvec = big.tile([128, 16], f32, tag="bvec")
        nc.sync.dma_start(out=bvec, in_=bvec_d[:, :])
        mask = big.tile([128, BI_S, R], bf16, tag="mask")
        m01_ap = m01_d[:]
        m01_b = bass.AP(tensor=m01_ap.tensor, offset=m01_ap.offset,
                        ap=[[0, 128]] + list(m01_ap.ap))
        nc.gpsimd.dma_start(out=mask, in_=m01_b)
        imgsin = big.tile([128, 3, D], bf16, tag="imgsin")
        for ti, (r0, p) in enumerate(irows):
            nc.sync.dma_start(out=imgsin[:p, ti, :], in_=imgs_d[r0:r0 + p, :])
        capm = big.tile([128, NT, 2], f32, tag="capm")
        nc.sync.dma_start(out=capm, in_=capm_d[:, :].rearrange("(t p) c -> p t c", p=128))
        capsin = big.tile([128, NT, D], bf16, tag="capsin")
        capsT = big.tile([128, DC, NCW], bf16, tag="capsT")
        for c in range(4):
            rows = slice(640 * c, 640 * c + 640)
            nc.sync.dma_start(
                out=capsin[:, 5 * c:5 * c + 5, :],
                in_=caps_d[rows, :].rearrange("(t p) d -> p t d", p=128))
            nc.sync.dma_start(
                out=capsT[:, :, rows],
                in_=capsT_d[:, rows].rearrange("(k p) n -> p k n", p=128))

        # ---------------- phase A: image side (per-core slice, 288 rows) ----
        lniT = big.tile([128, DC, IR], bf16, tag="lniT")
        for ti, (r0, p) in enumerate(irows):
            x = imgsin[:, ti, :]
            st = mvp.tile([128, 6], f32, tag="st")
            nc.vector.bn_stats(st[:p], x[:p])
            ag = mvp.tile([128, 2], f32, tag="ag")
            nc.vector.bn_aggr(ag[:p], st[:p])
            sg = mvp.tile([128, 1], f32, tag="sg")
            nc.scalar.activation(sg[:p], ag[:p, 1:2], AF.Sqrt, bias=epsi[:p])
            iv = mvp.tile([128, 1], f32, tag="iv")
            nc.vector.reciprocal(iv[:p], sg[:p])
            ln = wrk.tile([128, D], bf16, tag="ln")
            nc.vector.tensor_scalar(out=ln[:p], in0=x[:p],
                                    scalar1=ag[:p, 0:1], scalar2=iv[:p],
                                    op0=OP.subtract, op1=OP.mult)
            for j in range(DC):
                pt = pst.tile([128, 128], bf16, tag="tr")
                nc.tensor.transpose(pt[:, :p], ln[:p, 128 * j:128 * j + 128],
                                    ident[:p, :p])
                nc.scalar.copy(out=lniT[:, j, r0:r0 + p], in_=pt[:, :p])

        # K^T (+bk') -- d on partitions
        kT = big.tile([128, DC, IR], bf16, tag="kT")
        for j in range(DC):
            ps = psb.tile([128, IR], f32, tag="ps")
            for k in range(DC):
                nc.tensor.matmul(ps, lhsT=wsb["wk"][:, k, 128 * j:128 * j + 128],
                                 rhs=lniT[:, k, :], start=(k == 0), stop=(k == DC - 1))
            nc.scalar.activation(kT[:, j, :], ps, AF.Identity,
                                 bias=bvec[:, 4 + j:5 + j])

        # V natural -> row-center -> V'^T (+bvc)
        vpT = big.tile([128, DC, IR], bf16, tag="vpT")
        for ti, (r0, p) in enumerate(irows):
            ps = psb.tile([128, D], f32, tag="ps")
            for k in range(DC):
                nc.tensor.matmul(ps[:p], lhsT=lniT[:, k, r0:r0 + p],
                                 rhs=wsb["wv"][:, k, :], start=(k == 0), stop=(k == DC - 1))
            ms = mvp.tile([128, 1], f32, tag="ms")
            nc.vector.reduce_sum(ms[:p], ps[:p], axis=AX.X)
            mu = mvp.tile([128, 1], f32, tag="mu")
            nc.scalar.mul(mu[:p], ms[:p], 1.0 / D)
            vb = wrk.tile([128, D], bf16, tag="vb")
            nc.vector.tensor_scalar_sub(out=vb[:p], in0=ps[:p], scalar1=mu[:p])
            for j in range(DC):
                pt = pst.tile([128, 128], bf16, tag="tr")
                nc.tensor.transpose(pt[:, :p], vb[:p, 128 * j:128 * j + 128],
                                    ident[:p, :p])
                nc.scalar.activation(vpT[:, j, r0:r0 + p], pt[:, :p], AF.Identity,
                                     bias=bvec[:, 12 + j:13 + j])

        # P^T = Wo_g4 @ V'^T + pc
        pT = big.tile([128, DC, IR], bf16, tag="pT")
        for j in range(DC):
            ps = psb.tile([128, IR], f32, tag="ps")
            for k in range(DC):
                nc.tensor.matmul(ps, lhsT=wsb["wo"][:, k, 128 * j:128 * j + 128],
                                 rhs=vpT[:, k, :], start=(k == 0), stop=(k == DC - 1))
            nc.scalar.activation(pT[:, j, :], ps, AF.Identity,
                                 bias=bvec[:, 8 + j:9 + j])
            nc.vector.tensor_mul(pT[:, j, :], pT[:, j, :],
                                 mask[:, :, :].rearrange("p a b -> p (a b)"))

        # M Gram per image (duplicated at partition bases 0 and 64)
        sm = big.tile([128, BI_S, R], bf16, tag="sm")
        for i in range(BI_S):
            gps = pst.tile([36, 36], f32, tag="tr")
            for k in range(DC):
                nc.tensor.matmul(gps, lhsT=pT[:, k, R * i:R * i + R],
                                 rhs=pT[:, k, R * i:R * i + R],
                                 start=(k == 0), stop=(k == DC - 1))
            nc.scalar.copy(sm[0:R, i, :], gps)
            nc.scalar.copy(sm[64:64 + R, i, :], gps)

        # ---------------- phase B: caption stats + centered Q^T projection --
        qT = big.tile([128, DC, NCW], bf16, tag="qT")
        isv = big.tile([128, NT], f32, tag="isv")
        muRow = big.tile([1, NCW], bf16, tag="muRow")
        agAll = big.tile([128, NT, 2], f32, tag="agAll")
        for t in range(NT):
            st = mvp.tile([128, 6], f32, tag="st")
            nc.vector.bn_stats(st, capsin[:, t, :])
            nc.vector.bn_aggr(agAll[:, t, :], st)
            mb_ = mvp.tile([128, 1], bf16, tag="mb")
            nc.vector.tensor_copy(mb_, agAll[:, t, 0:1])
            pt = pst.tile([128, 128], bf16, tag="tr")
            nc.tensor.transpose(pt[0:1, :], mb_, ident)
            nc.vector.tensor_copy(muRow[0:1, 128 * t:128 * t + 128], pt[0:1, :])
        sgA = big.tile([128, NT], f32, tag="sgA")
        nc.scalar.activation(sgA, agAll[:, :, 1], AF.Sqrt, bias=epsc,
                             scale=float(D))
        nc.vector.reciprocal(isv, sgA)
        for c in range(NT // 4):
            for j in range(DC):
                ps = psb.tile([128, 512], f32, tag="ps")
                for k in range(DC):
                    nc.tensor.matmul(ps, lhsT=wsb["wq"][:, k, 128 * j:128 * j + 128],
                                     rhs=capsT[:, k, 512 * c:512 * c + 512],
                                     start=(k == 0), stop=False)
                nc.tensor.matmul(ps, lhsT=wg1row[0:1, 128 * j:128 * j + 128],
                                 rhs=muRow[0:1, 512 * c:512 * c + 512],
                                 start=False, stop=True)
                nc.scalar.activation(qT[:, j, 512 * c:512 * c + 512], ps,
                                     AF.Identity, bias=bvec[:, j:j + 1])

        # ---------------- phase C: per caption-word tile ---------------------
        nTall = big.tile([128, NT, BI_S], f32, tag="nTall")
        nMall = big.tile([128, NT, BI_S], f32, tag="nMall")
        for t in range(NT):
            cw = slice(128 * t, 128 * t + 128)
            psK = psb.tile([128, BI_S, R], f32, tag="ps")
            psT = psb.tile([128, BI_S, R], f32, tag="ps")
            for k in range(DC):
                nc.tensor.matmul(psK, lhsT=qT[:, k, cw], rhs=kT[:, k, :],
                                 start=(k == 0), stop=(k == DC - 1))
            for k in range(DC):
                nc.tensor.matmul(psT, lhsT=qT[:, k, cw], rhs=pT[:, k, :],
                                 start=(k == 0), stop=(k == DC - 1))
            A = apl.tile([128, BI_S, R], bf16, tag="A")
            nc.scalar.activation(A, psK, AF.Exp, scale=isv[:, t:t + 1])

            ats = []
            for i in range(BI_S):
                pt = pst.tile([128, 128], bf16, tag="tr")
                nc.tensor.transpose(pt[:R, :], A[:, i, :], ident)
                at = atp.tile([64, 128], bf16, tag="at")
                if i % 2 == 0:
                    nc.vector.tensor_copy(at[:R, :], pt[:R, :])
                else:
                    nc.scalar.copy(at[:R, :], pt[:R, :])
                ats.append(at)
            psB = psb.tile([128, BI_S, R], f32, tag="ps")
            for i in range(BI_S):
                nc.tensor.matmul(psB[:, i, :], lhsT=ats[i][:R, :],
                                 rhs=sm[0:R, i, :], start=True, stop=True)

            scT = scr.tile([128, BI_S, R], bf16, tag="scT")
            nc.vector.tensor_mul(scT, A, psT)
            nc.vector.reduce_sum(nTall[:, t, :], scT, axis=AX.X)
            scM = scr.tile([128, BI_S, R], bf16, tag="scM")
            nc.vector.tensor_mul(scM, A, psB)
            nc.vector.reduce_sum(nMall[:, t, :], scM, axis=AX.X)

        # ---------------- batched epilogue ----------------------------------
        def bcast8(ap2d):
            return bass.AP(tensor=ap2d.tensor, offset=ap2d.offset,
                           ap=list(ap2d.ap) + [[0, BI_S]])

        sqM = big.tile([128, NT, BI_S], f32, tag="sqM")
        nc.scalar.activation(sqM, nMall, AF.Sqrt)
        nc.gpsimd.tensor_scalar_add(sqM, sqM, 1e-12)
        rr = big.tile([128, NT, BI_S], f32, tag="rr")
        nc.vector.reciprocal(rr, sqM)
        nTs = big.tile([128, NT, BI_S], f32, tag="nTs")
        nc.gpsimd.tensor_mul(nTs, nTall, bcast8(isv[:, :]))
        s0 = big.tile([128, NT, BI_S], f32, tag="s0")
        nc.gpsimd.tensor_mul(s0, nTs, rr)
        s1 = big.tile([128, NT, BI_S], f32, tag="s1")
        nc.gpsimd.tensor_mul(s1, s0, bcast8(capm[:, :, 0]))
        sf = big.tile([128, BI_S, NT], f32, tag="sf")
        nc.gpsimd.tensor_add(sf[:, :, :].rearrange("p i t -> p t i"), s1,
                             bcast8(capm[:, :, 1]))
        if "tiledma" in FL:
            for t in range(NT):
                nc.sync.dma_start(
                    out=out_d[:, 128 * t:128 * t + 128].rearrange("i p -> p i"),
                    in_=sf[:, :, t])
        else:
            nc.sync.dma_start(
                out=out_d[:, :].rearrange("i (t p) -> p (i t)", p=128), in_=sf)

    nc.finalize()
    return nc


def _get_program():
    if "nc" not in _PROG_CACHE:
        _PROG_CACHE["nc"] = _build_program()
    return _PROG_CACHE["nc"]


# ------------------------------------------------------------------- driver --
def kernel(imgs, caps, img_lens, cap_lens,
           Wq, bq, Wk, bk, Wv, bv, Wo, bo,
           g1, b1, g2, b2, g3, b3, g4, b4):
    global LAST_EXEC_NS, LAST_TRACE
    args = dict(imgs=imgs, caps=caps, img_lens=img_lens, cap_lens=cap_lens,
                Wq=Wq, bq=bq, Wk=Wk, bk=bk, Wv=Wv, bv=bv, Wo=Wo, bo=bo,
                g1=g1, b1=b1, g2=g2, b2=b2, g3=g3, b3=b3, g4=g4, b4=b4)
    args = {k: np.asarray(v, np.float32) if np.asarray(v).dtype != np.int32
            else np.asarray(v) for k, v in args.items()}
    imgs, caps = args["imgs"], args["caps"]
    img_lens, cap_lens = np.asarray(img_lens, np.int32), np.asarray(cap_lens, np.int32)
    c0 = args["Wo"] @ args["b4"] + args["bo"]
    qb0 = args["Wq"] @ args["b1"] + args["bq"]
    if ((imgs.shape, caps.shape) != ((Bi, R, D), (Bc, W, D))
            or np.abs(c0).max() != 0 or np.abs(qb0).max() != 0):
        return _np_kernel(**args)
    try:
        return _device_kernel(args, img_lens, cap_lens)
    except Exception:
        import traceback
        traceback.print_exc()
        print("kernel: device path failed; falling back to numpy", file=sys.stderr)
        return _np_kernel(**args)


def _device_kernel(a, img_lens, cap_lens):
    global LAST_EXEC_NS, LAST_TRACE
    import ml_dtypes
    from concourse.bass_utils import run_bass_kernel_spmd

    bf = ml_dtypes.bfloat16
    img_valid = (np.arange(R)[None, :] < img_lens[:, None])
    cap_valid = (np.arange(W)[None, :] < cap_lens[:, None])
    imgs_m = (a["imgs"] * img_valid[..., None]).reshape(Bi * R, D)
    caps_m = (a["caps"] * cap_valid[..., None]).reshape(NCW, D)

    Wq_g = a["Wq"] * a["g1"][None, :]
    WqgT = np.ascontiguousarray(Wq_g.T).astype(bf)
    WkgT = np.ascontiguousarray((a["Wk"] * a["g2"][None, :]).T).astype(bf)
    WvgT = np.ascontiguousarray((a["Wv"] * a["g3"][None, :]).T).astype(bf)
    WogT = np.ascontiguousarray((a["Wo"] * a["g4"][None, :]).T).astype(bf)
    wg1n = np.ascontiguousarray(-Wq_g.sum(axis=1)).astype(bf)
    qb = a["Wq"] @ a["b1"] + a["bq"]
    bk_ = a["Wk"] @ a["b2"] + a["bk"]
    bv_ = a["Wv"] @ a["b3"] + a["bv"]
    bvc = (bv_ - bv_.mean()).astype(np.float32)
    pc = (a["Wo"] * a["g4"][None, :]) @ bvc
    bvec = np.stack([qb.reshape(DC, 128), bk_.reshape(DC, 128),
                     pc.reshape(DC, 128), bvc.reshape(DC, 128)],
                    axis=0).reshape(16, 128).T
    bvec = np.ascontiguousarray(bvec, dtype=np.float32)  # (128, 16)

    capm = cap_valid.reshape(NCW, 1).astype(np.float32)
    capm2 = np.ascontiguousarray(
        np.concatenate([capm * np.float32(np.sqrt(D)), capm - 1.0], axis=1))

    caps_bf = np.ascontiguousarray(caps_m).astype(bf)
    capsT_bf = np.ascontiguousarray(caps_m.T).astype(bf)
    in_maps = []
    for c in range(N_CORES):
        sl = slice(c * BI_S * R, (c + 1) * BI_S * R)
        in_maps.append({
            "caps": caps_bf,
            "capsT": capsT_bf,
            "imgs": np.ascontiguousarray(imgs_m[sl]).astype(bf),
            "wq": WqgT, "wk": WkgT, "wv": WvgT, "wo": WogT,
            "wg1n": wg1n,
            "bvec": bvec,
            "mask01": np.ascontiguousarray(
                img_valid[c * BI_S:(c + 1) * BI_S].reshape(IR)).astype(bf),
            "capm2": capm2,
        })

    nc = _get_program()
    trace = bool(os.environ.get("BASS_KTRACE"))
    kw = {}
    if trace:
        kw = dict(trace=True, tmpdir=os.environ.get("BASS_KTRACE_DIR") or None)
    res = run_bass_kernel_spmd(nc, in_maps, list(range(N_CORES)), **kw)
    if trace:
        LAST_EXEC_NS = res.exec_time_ns
        LAST_TRACE = res.profile_json
    out = np.concatenate(
        [r["out"].reshape(BI_S, Bc, W) for r in res.results], axis=0)
    return np.ascontiguousarray(out.astype(np.float32))
